# revision 24
# baseline (speedup 1.0000x reference)
"""Trainium2 Bass kernel for nn_Attention_83081847374268 (sparse sliding-window GQA).

Sharding: 8 cores = batch (2, data parallel) x kv-head (4, tensor parallel).
Each core computes, for its (b, kh): q/k/v projections (2 q heads, 1 kv head),
QK-RMSNorm + RoPE, banded sliding-window attention, and a partial output
projection against its 512-row slice of wout.  The host sums the 4 partials
per batch (the TP reduction) and stacks the batches.

v2 design (vs the fp32r baseline):
  * all DMA'd tensors are bf16 (x, weights, rope tables, bias, output) --
    halves HBM traffic and enables fast weight loads (FWL) on the PE;
    PSUM accumulation stays fp32 so only operand rounding is lost.
  * v is computed directly in natural [t, head_dim] layout (x-chunk as the
    stationary operand) into a persistent SBUF buffer -- no DRAM bounce, no
    PE transposes for v.
  * the three stages are interleaved per 512-token chunk:
      A(chunk t) -> B(groups 2t, 2t+1) -> C(out-proj tiles 4t..4t+3)
    so the PE never idles long enough for HAM to re-throttle and all DMA
    (x in, yp out) streams concurrently with compute.
  * bias (mask) tiles are deduped host-side (interior tiles share one banded
    pattern) and kept resident in SBUF.
  * kT / v_sb are zeroed up front so tiles whose padded key window reaches
    past the currently-written columns read zeros (masked to exp(-100)=0),
    never junk/NaN.
"""
import sys

sys.path.insert(0, "/opt/trn_rl_repo")

import numpy as np
import ml_dtypes

import concourse.bacc as bacc
import concourse.mybir as mybir
from concourse.bass_utils import run_bass_kernel_spmd
from concourse.tile import TileContext
from concourse.alu_op_type import AluOpType

F32 = mybir.dt.float32
F32R = mybir.dt.float32r
BF16 = mybir.dt.bfloat16
ACTF = mybir.ActivationFunctionType

B, T, WIDTH = 2, 2048, 2048
NUM_HEADS, NUM_KV_HEADS, HEAD_DIM = 8, 4, 256
GROUPS = NUM_HEADS // NUM_KV_HEADS  # 2 q heads per kv head (= per core)
WINDOW = 512
ROPE_BASE = 10000.0
MASK_NEG = -100.0  # exp(S/16 + MASK_NEG) == 0 for |S|<=~16; exact in bf16

NT = T // 128           # 16 query tiles
TCH = 512               # stage-A t-chunk width
NTCH = T // TCH         # 4
NW = WIDTH // 128       # 16 contraction chunks

_prog_cache = {}


def _round_up(x, m):
    return (x + m - 1) // m * m


def _geometry(positions, attn_mask):
    """Per-query-tile key windows from the actual mask/positions data."""
    pos = np.asarray(positions)
    am = np.asarray(attn_mask)
    pd = pos[:, :, None].astype(np.int64) - pos[:, None, :].astype(np.int64)
    valid = am & (np.abs(pd) < WINDOW)  # [B, T, T] bool
    assert valid.any(axis=2).all(), "a query row with no valid key is unsupported"
    js = []
    whi = []
    wmax = 0
    for it in range(NT):
        cols = valid[:, it * 128:(it + 1) * 128, :].any(axis=(0, 1))
        idx = np.nonzero(cols)[0]
        j_lo, j_hi = int(idx[0]), int(idx[-1]) + 1
        j0 = (j_lo // 128) * 128
        wmax = max(wmax, j_hi - j0)
        js.append(j0)
        whi.append(j_hi)
    Wb = max(256, _round_up(wmax, 128))
    Wb = min(Wb, T)
    js = tuple(max(0, min(j, T - Wb)) for j in js)
    wext = tuple(min(Wb, _round_up(whi[it] - js[it], 128)) for it in range(NT))
    return valid, Wb, js, wext


def _s_pieces(Wb):
    """Split Wb into PSUM-bank-aligned moving pieces (<=512 fp32 per bank)."""
    out = []
    rem = Wb
    while rem > 0:
        out.append(min(512, rem))
        rem -= out[-1]
    return out


def _rope_tables(pos_b, scale):
    """cos/sin tables in [head_dim/2, T] (transposed) layout, gain folded in."""
    d = np.arange(HEAD_DIM // 2, dtype=np.float32)
    timescale = (ROPE_BASE ** (2.0 / HEAD_DIM * d)).astype(np.float32)
    rad = pos_b.astype(np.float32)[None, :] / timescale[:, None]  # [128, T]
    cos, sin = np.cos(rad).astype(np.float32), np.sin(rad).astype(np.float32)
    g1 = (1.0 + scale[:HEAD_DIM // 2]).astype(np.float32)[:, None]
    g2 = (1.0 + scale[HEAD_DIM // 2:]).astype(np.float32)[:, None]
    # o1 = a1*C1 - a2*S2 ; o2 = a2*C2 + a1*S1
    return (cos * g1, sin * g1, cos * g2, sin * g2)  # C1, S1, C2, S2


def _build(Wb, js, wext, n_bias, bias_map, shared_tables):
    nc = bacc.Bacc("TRN2", target_bir_lowering=False, debug=False, num_devices=8)

    def din(name, shape, dt):
        return nc.dram_tensor(name, shape, dt, kind="ExternalInput").ap()

    xT = din("xT", [WIDTH, T], BF16)
    wq = din("wq", [WIDTH, 512], BF16)
    wk = din("wk", [WIDTH, 256], BF16)
    wv = din("wv", [WIDTH, 256], BF16)
    wout = din("wout", [512, T], BF16)
    ident_d = din("ident", [128, 128], BF16)
    ones1_d = din("ones1", [1, 128], F32R)    # K=1 broadcast lhsT
    onesc_d = din("onesc", [128, 1], F32R)    # partition-sum lhsT
    bias_d = din("bias", [n_bias, 128, Wb], BF16)
    tab_names = ["ct", "st"] if shared_tables else [
        "cq1", "sq1", "cq2", "sq2", "ck1", "sk1", "ck2", "sk2"]
    tabs = {n: din(n, [128, T], BF16) for n in tab_names}
    yp = nc.dram_tensor("yp", [T, T], BF16, kind="ExternalOutput").ap()

    NJ = Wb // 128
    # per-group (2 query tiles) union of key chunks, and which halves exist
    groups = []
    for g in range(NT // 2):
        w0 = set(range(js[2 * g] // 128,
                       js[2 * g] // 128 + wext[2 * g] // 128))
        w1 = set(range(js[2 * g + 1] // 128,
                       js[2 * g + 1] // 128 + wext[2 * g + 1] // 128))
        groups.append([(jc, jc in w0, jc in w1) for jc in sorted(w0 | w1)])

    with TileContext(nc) as tc:
        with (
            tc.tile_pool(name="persist", bufs=1) as pp,
            tc.tile_pool(name="qk_store", bufs=1) as qkp,
            tc.tile_pool(name="wpool", bufs=1) as wp,
            tc.tile_pool(name="xpool", bufs=2) as xp,
            tc.tile_pool(name="sa", bufs=1) as sa,
            tc.tile_pool(name="sb2", bufs=2) as sb2,
            tc.tile_pool(name="ptp", bufs=2) as ptp,
            tc.tile_pool(name="outp", bufs=2) as outp,
            tc.tile_pool(name="ps_pair", bufs=2, space="PSUM") as ps_pair,
            tc.tile_pool(name="ps_aux", bufs=2, space="PSUM") as ps_aux,
            tc.tile_pool(name="ps_o", bufs=2, space="PSUM") as ps_o,
        ):
            # ---------- persistent SBUF state ----------
            qT = [qkp.tile([128, T], BF16, tag=f"qT{c}", name=f"qT{c}") for c in range(4)]
            kT = [qkp.tile([128, T], BF16, tag=f"kT{c}", name=f"kT{c}") for c in range(2)]
            v_sb = qkp.tile([128, NT * 256], BF16, tag="v_sb", name="v_sb")
            encT = [qkp.tile([128, T], BF16, tag=f"encT{c}", name=f"encT{c}")
                    for c in range(4)]
            wq_t = wp.tile([128, NW * 512], BF16)
            wk_t = wp.tile([128, NW * 256], BF16)
            wv_t = wp.tile([128, NW * 256], BF16)
            wq_r = wq.rearrange("(c p) m -> p c m", p=128)
            wk_r = wk.rearrange("(c p) m -> p c m", p=128)
            wv_r = wv.rearrange("(c p) m -> p c m", p=128)
            wq_v = wq_t[:].rearrange("p (c m) -> p c m", m=512)
            wk_v = wk_t[:].rearrange("p (c m) -> p c m", m=256)
            wv_v = wv_t[:].rearrange("p (c m) -> p c m", m=256)
            xT_r = xT.rearrange("(c p) t -> p c t", p=128)

            # first DMAs in the queue: wk + x chunk 0, finely chunked so the
            # first unit's matmuls start ASAP
            xts0 = xp.tile([128, NW * TCH], BF16, tag="xts", name="xts0")
            xv0 = xts0[:].rearrange("p (c t) -> p c t", t=TCH)
            for wc in range(4):
                nc.sync.dma_start(out=wk_v[:, wc:wc + 1], in_=wk_r[:, wc:wc + 1])
                nc.sync.dma_start(out=xv0[:, wc:wc + 1],
                                  in_=xT_r[:, wc:wc + 1, 0:TCH])
            for q4 in range(1, 4):
                nc.sync.dma_start(out=wk_v[:, q4 * 4:(q4 + 1) * 4],
                                  in_=wk_r[:, q4 * 4:(q4 + 1) * 4])
                nc.sync.dma_start(out=xv0[:, q4 * 4:(q4 + 1) * 4],
                                  in_=xT_r[:, q4 * 4:(q4 + 1) * 4, 0:TCH])
            for q4 in range(4):
                nc.scalar.dma_start(out=wq_v[:, q4 * 4:(q4 + 1) * 4],
                                    in_=wq_r[:, q4 * 4:(q4 + 1) * 4])

            # PE warmup: ~4.5us of dummy matmuls on a zeroed tile gets the
            # HAM clock-gate to full rate before the real matmuls arrive.
            # The result (zeros) is written back into zero_b, which IS
            # consumed later, so the chain is not dead code.
            zero_b = pp.tile([128, 128], BF16)
            nc.any.memset(zero_b[:], 0.0)
            warm_ps = ps_o.tile([128, 128], F32, tag="t_po", name="warm_ps")
            for _ in range(120):
                nc.tensor.matmul(warm_ps[:], zero_b[:], zero_b[:],
                                 start=True, stop=True)
            nc.vector.tensor_copy(zero_b[:], warm_ps[:])

            for c in range(2):
                nc.any.memset(kT[c][:], 0.0)
            nc.any.memset(v_sb[:], 0.0)

            ident = pp.tile([128, 128], BF16)
            nc.scalar.dma_start(out=ident[:], in_=ident_d[:])
            ones1 = pp.tile([1, 128], F32R)
            nc.scalar.dma_start(out=ones1[:], in_=ones1_d[:])
            onesc = pp.tile([128, 1], F32R)
            nc.scalar.dma_start(out=onesc[:], in_=onesc_d[:])
            epsb = pp.tile([1, 1], F32)
            nc.any.memset(epsb[:], 1e-6)
            epsbq = pp.tile([1, 1], F32)
            nc.any.memset(epsbq[:], HEAD_DIM * 1e-6)
            ones_f = pp.tile([1, 1], F32)
            nc.any.memset(ones_f[:], 1.0)
            rstdq_c = [pp.tile([128, NT], F32, tag=f"rstdq{hh}", name=f"rstdq{hh}")
                       for hh in range(2)]

            # ---------- remaining prologue loads ------
            def load_xts(tci):
                t0 = tci * TCH
                xts = xp.tile([128, NW * TCH], BF16, tag="xts", name=f"xts{tci}")
                xv = xts[:].rearrange("p (c t) -> p c t", t=TCH)
                for q4 in range(4):
                    nc.sync.dma_start(
                        out=xv[:, q4 * 4:(q4 + 1) * 4],
                        in_=xT_r[:, q4 * 4:(q4 + 1) * 4, t0:t0 + TCH],
                    )
                return xts

            tabt = {}
            for name in tab_names:
                tt = pp.tile([128, T], BF16, tag=name, name=f"tab_{name}")
                nc.sync.dma_start(out=tt[:], in_=tabs[name][:])
                tabt[name] = tt
            for q4 in range(4):
                nc.scalar.dma_start(out=wv_v[:, q4 * 4:(q4 + 1) * 4],
                                    in_=wv_r[:, q4 * 4:(q4 + 1) * 4])
            bias_t = []
            for bi in range(n_bias):
                bt = pp.tile([128, Wb], BF16, tag=f"bias{bi}", name=f"bias{bi}")
                nc.scalar.dma_start(out=bt[:], in_=bias_d[bi])
                bias_t.append(bt)
            wout_t = [pp.tile([128, T], BF16, tag=f"wo{c}", name=f"wo{c}")
                      for c in range(4)]
            wout_r = wout.rearrange("(c p) t -> c p t", p=128)
            for c in range(4):
                nc.scalar.dma_start(out=wout_t[c][:], in_=wout_r[c])

            if shared_tables:
                q_tabs = k_tabs = ("ct", "st", "ct", "st")
            else:
                q_tabs = ("cq1", "sq1", "cq2", "sq2")
                k_tabs = ("ck1", "sk1", "ck2", "sk2")
            units = [
                (wk_t, 256, 0, k_tabs, kT, 0, None),
                (wq_t, 512, 0, q_tabs, qT, 0, 0),
                (wq_t, 512, 256, q_tabs, qT, 2, 1),
            ]

            # ---------------- stage A: projections + RMSNorm + RoPE ----------
            def stage_a(tci, xts):
                t0 = tci * TCH
                for w_t, wcols, cbase, tkeys, dest, dbase, qhead in units:
                    pspair = ps_pair.tile([128, 2 * TCH], F32, tag="pspair",
                                          name=f"pjp{tci}_{dbase}_{qhead}")
                    ps1 = pspair[:, 0:TCH]
                    ps2 = pspair[:, TCH:2 * TCH]
                    for ps, cc in ((ps1, 0), (ps2, 1)):
                        coff = cbase + cc * 128
                        for wc in range(NW):
                            nc.tensor.matmul(
                                ps,
                                w_t[:, wc * wcols + coff: wc * wcols + coff + 128],
                                xts[:, wc * TCH:(wc + 1) * TCH],
                                start=(wc == 0), stop=(wc == NW - 1),
                            )
                    sq1 = sa.tile([128, TCH], F32R, tag="sq1")
                    sq2 = sa.tile([128, TCH], F32R, tag="sq2")
                    nc.scalar.activation(sq1[:], ps1, ACTF.Square)
                    nc.scalar.activation(sq2[:], ps2, ACTF.Square)
                    psvar = ps_aux.tile([1, TCH], F32, tag="t_aux",
                                        name=f"pvar{tci}_{dbase}_{qhead}")
                    nc.tensor.matmul(psvar[:], onesc[:], sq1[:], start=True, stop=False)
                    nc.tensor.matmul(psvar[:], onesc[:], sq2[:], start=False, stop=True)
                    C1, S1, C2, S2 = (tabt[k][:, t0:t0 + TCH] for k in tkeys)
                    m1 = sa.tile([128, TCH], F32, tag="m1")
                    m2 = sa.tile([128, TCH], F32, tag="m2")
                    m3 = sa.tile([128, TCH], F32, tag="m1", name="m3t")
                    m4 = sa.tile([128, TCH], F32, tag="m2", name="m4t")
                    if qhead is None:
                        # k: apply rstd via PE broadcast, fused into rope
                        stdv = sa.tile([1, TCH], F32R, tag="stdv")
                        nc.scalar.activation(stdv[:], psvar[:], ACTF.Sqrt,
                                             scale=1.0 / HEAD_DIM, bias=epsb[:])
                        psb = ps_aux.tile([128, TCH], F32, tag="t_aux",
                                          name=f"psb{tci}")
                        nc.tensor.matmul(psb[:], ones1[:], stdv[:],
                                         start=True, stop=True)
                        rb = sa.tile([128, TCH], F32, tag="rb")
                        nc.vector.reciprocal_approx_fast(out=rb[:], in_=psb[:])
                        a1 = sa.tile([128, TCH], F32, tag="a1")
                        a2 = sa.tile([128, TCH], F32, tag="a2")
                        nc.vector.tensor_tensor(a1[:], ps1, rb[:], AluOpType.mult)
                        nc.vector.tensor_tensor(a2[:], ps2, rb[:], AluOpType.mult)
                    else:
                        # q: defer 1/std to the stage-B logits scale;
                        # transpose 4*std per 128-tile via K=1 matmuls
                        stdvf = sa.tile([1, TCH], F32, tag="stdvf")
                        nc.scalar.activation(stdvf[:], psvar[:], ACTF.Sqrt,
                                             bias=epsbq[:])
                        sq_ps = ps_aux.tile([128, TCH // 128], F32, tag="t_aux",
                                            name=f"sqps{tci}_{qhead}")
                        for s in range(TCH // 128):
                            nc.tensor.matmul(
                                sq_ps[:, s:s + 1],
                                stdvf[:, s * 128:(s + 1) * 128],
                                ones_f[:], start=True, stop=True)
                        stdq = sa.tile([128, TCH // 128], F32, tag="stdq")
                        nc.scalar.activation(stdq[:], sq_ps[:], ACTF.Copy)
                        nc.vector.reciprocal_approx_fast(
                            out=rstdq_c[qhead][:, tci * (TCH // 128):
                                               (tci + 1) * (TCH // 128)],
                            in_=stdq[:])
                        a1, a2 = ps1, ps2
                    nc.vector.tensor_tensor(m1[:], a1, C1, AluOpType.mult)
                    nc.vector.tensor_tensor(m2[:], a2, S2, AluOpType.mult)
                    nc.vector.tensor_tensor(
                        dest[dbase][:, t0:t0 + TCH], m1[:], m2[:], AluOpType.subtract)
                    nc.vector.tensor_tensor(m3[:], a2, C2, AluOpType.mult)
                    nc.vector.tensor_tensor(m4[:], a1, S1, AluOpType.mult)
                    nc.vector.tensor_tensor(
                        dest[dbase + 1][:, t0:t0 + TCH], m3[:], m4[:], AluOpType.add)
                # v in natural [t, 256] layout: x-chunk stationary, wv moving
                for s in range(TCH // 128):
                    psv = ps_o.tile([128, 256], F32, tag="t_po",
                                    name=f"psv{tci}_{s}")
                    for wc in range(NW):
                        nc.tensor.matmul(
                            psv[:],
                            xts[:, wc * TCH + s * 128: wc * TCH + (s + 1) * 128],
                            wv_t[:, wc * 256:(wc + 1) * 256],
                            start=(wc == 0), stop=(wc == NW - 1),
                        )
                    jc = tci * (TCH // 128) + s
                    nc.vector.tensor_copy(v_sb[:, jc * 256:(jc + 1) * 256], psv[:])

            # ---------------- stage B: banded attention ----------------------
            def stage_b(g):
                ginfo = groups[g]
                nj = len(ginfo)
                jc0 = ginfo[0][0]
                pts_all = ptp.tile([128, nj * 512], BF16, tag="pts", name=f"pts{g}")
                for i, (jc, inA, inB) in enumerate(ginfo):
                    for h in range(2):
                        if not inA:
                            nc.vector.tensor_copy(
                                pts_all[:, i * 512 + h * 256:
                                        i * 512 + h * 256 + 128], zero_b[:])
                        if not inB:
                            nc.vector.tensor_copy(
                                pts_all[:, i * 512 + h * 256 + 128:
                                        i * 512 + h * 256 + 256], zero_b[:])
                den2 = sb2.tile([128, 2 * 2], F32, tag="den2", name=f"den2_{g}")
                pdict = {}
                for half in range(2):
                    it = 2 * g + half
                    jst = js[it]
                    wx = wext[it]
                    nj_t = wx // 128
                    bt = bias_t[bias_map[it]]
                    t_pieces = _s_pieces(wx)
                    for h in range(2):
                        S_ps = ps_pair.tile([128, 2 * TCH], F32, tag="pspair",
                                            name=f"S{it}_{h}")
                        for cc in range(2):
                            col = 0
                            for pi, pw in enumerate(t_pieces):
                                nc.tensor.matmul(
                                    S_ps[:, pi * 512: pi * 512 + pw],
                                    qT[2 * h + cc][:, it * 128:(it + 1) * 128],
                                    kT[cc][:, jst + col: jst + col + pw],
                                    start=(cc == 0), stop=(cc == 1),
                                )
                                col += pw
                        S_b = sb2.tile([128, Wb], BF16, tag="Sb", name=f"Sb{it}_{h}")
                        nc.vector.scalar_tensor_tensor(
                            S_b[:, 0:wx], S_ps[:, 0:wx],
                            rstdq_c[h][:, it:it + 1],
                            bt[:, 0:wx],
                            AluOpType.mult, AluOpType.add)
                        P_t = sb2.tile([128, Wb], BF16, tag=f"P{h}", name=f"P{it}_{h}")
                        nc.scalar.activation(
                            P_t[:, 0:wx], S_b[:, 0:wx], ACTF.Exp,
                            accum_out=den2[:, half * 2 + h: half * 2 + h + 1])
                        pdict[(half, h)] = P_t
                    for h in range(2):
                        rden = sb2.tile([128, 1], F32, tag=f"rden{h}",
                                        name=f"rden{it}_{h}")
                        nc.vector.reciprocal_approx_fast(
                            out=rden[:],
                            in_=den2[:, half * 2 + h: half * 2 + h + 1])
                        P_t = pdict[(half, h)]
                        Pn = sb2.tile([128, Wb], BF16, tag="Pn", name=f"Pn{it}_{h}")
                        nc.vector.tensor_scalar_mul(Pn[:, 0:wx], P_t[:, 0:wx],
                                                    rden[:])
                        idx0 = next(i for i, (c, _, _) in enumerate(ginfo)
                                    if c == jst // 128)
                        ps_t = ps_aux.tile([128, NJ * 128], BF16, tag="t_aux",
                                           name=f"ptps{it}_{h}")
                        for k in range(nj_t):
                            nc.tensor.transpose(
                                ps_t[:, k * 128:(k + 1) * 128],
                                Pn[:, k * 128:(k + 1) * 128],
                                ident[:])
                        pts_v = pts_all[:].rearrange(
                            "p (i f c) -> p i f c", f=4, c=128)
                        nc.vector.tensor_copy(
                            pts_v[:, idx0: idx0 + nj_t, h * 2 + half, :],
                            ps_t[:, 0:nj_t * 128].rearrange(
                                "p (k c) -> p k c", c=128))
                # PV: accumulate encoded^T from v_sb directly
                for cc in range(2):
                    eps = ps_o.tile([128, 512], F32, tag="t_po", name=f"eps{g}_{cc}")
                    for i, (jc, _, _) in enumerate(ginfo):
                        nc.tensor.matmul(
                            eps[:], v_sb[:, jc * 256 + cc * 128: jc * 256 + (cc + 1) * 128],
                            pts_all[:, i * 512:(i + 1) * 512],
                            start=(i == 0), stop=(i == nj - 1),
                        )
                    for h in range(2):
                        nc.scalar.activation(
                            encT[2 * h + cc][:, g * 256:(g + 1) * 256],
                            eps[:, h * 256:(h + 1) * 256], ACTF.Copy)

            # ---------------- stage C: output projection ----------------------
            def stage_c(tt):
                ob = outp.tile([128, T], BF16, tag="ob", name=f"ob{tt}")
                for nb in range(4):
                    ops = ps_o.tile([128, 512], F32, tag="t_po", name=f"ops{tt}_{nb}")
                    for cc in range(4):
                        nc.tensor.matmul(
                            ops[:],
                            encT[cc][:, tt * 128:(tt + 1) * 128],
                            wout_t[cc][:, nb * 512:(nb + 1) * 512],
                            start=(cc == 0), stop=(cc == 3),
                        )
                    nc.vector.tensor_copy(ob[:, nb * 512:(nb + 1) * 512], ops[:])
                    nc.sync.dma_start(
                        out=yp[tt * 128:(tt + 1) * 128, nb * 512:(nb + 1) * 512],
                        in_=ob[:, nb * 512:(nb + 1) * 512])

            # ---------------- interleaved main loop --------------------------
            # stage C trails one group so its matmuls are ready PE work while
            # the next group's softmax (DVE/ACT) chains run
            for tci in range(NTCH):
                xts = xts0 if tci == 0 else load_xts(tci)
                stage_a(tci, xts)
                for g in (2 * tci, 2 * tci + 1):
                    stage_b(g)
                    if g >= 1:
                        stage_c(2 * g - 2)
                        stage_c(2 * g - 1)
            stage_c(2 * (NT // 2) - 2)
            stage_c(2 * (NT // 2) - 1)

    nc.compile()
    return nc


def kernel(x, positions, attn_mask, wq, wkv, wout, q_scale, k_scale):
    x = np.asarray(x, np.float32)
    positions = np.asarray(positions)
    wq = np.asarray(wq, np.float32)
    wkv = np.asarray(wkv, np.float32)
    wout = np.asarray(wout, np.float32)
    q_scale = np.asarray(q_scale, np.float32)
    k_scale = np.asarray(k_scale, np.float32)

    valid, Wb, js, wext = _geometry(positions, attn_mask)
    shared = not (q_scale.any() or k_scale.any())

    # host-side bias bands: 0 where valid, MASK_NEG elsewhere (incl. padding);
    # identical tiles dedup to shared SBUF-resident tables
    bias = np.full((B, NT, 128, Wb), MASK_NEG, np.float32)
    for it in range(NT):
        j0 = js[it]
        w = min(Wb, T - j0)
        vslab = valid[:, it * 128:(it + 1) * 128, j0:j0 + w]
        bias[:, it, :, :w][vslab] = 0.0
    bias = bias.astype(ml_dtypes.bfloat16)
    bias_tabs = {}   # per batch: bytes -> idx
    bias_maps = []
    bias_lists = []
    for b in range(B):
        tab, bmap, blist = {}, [], []
        for it in range(NT):
            key = bias[b, it].tobytes()
            if key not in tab:
                tab[key] = len(blist)
                blist.append(bias[b, it])
            bmap.append(tab[key])
        bias_maps.append(tuple(bmap))
        bias_lists.append(np.stack(blist))
    # all cores of a batch share geometry; builds are keyed per distinct map
    keys = [(Wb, js, wext, len(bias_lists[b]), bias_maps[b], shared)
            for b in range(B)]
    for key in keys:
        if key not in _prog_cache:
            _prog_cache[key] = _build(*key)

    ident = np.eye(128, dtype=ml_dtypes.bfloat16)
    ones1 = np.ones((1, 128), np.float32)
    onesc = np.ones((128, 1), np.float32)

    def b16(a):
        return np.ascontiguousarray(a.astype(ml_dtypes.bfloat16))

    in_maps = []
    for core in range(8):
        b, kh = divmod(core, NUM_KV_HEADS)
        m = {
            "xT": b16(x[b].T),
            "wq": b16(wq[:, kh * 512:(kh + 1) * 512]),
            "wk": b16(wkv[:, kh * 256:(kh + 1) * 256]),
            "wv": b16(wkv[:, 1024 + kh * 256: 1024 + (kh + 1) * 256]),
            "wout": b16(wout[kh * 512:(kh + 1) * 512, :]),
            "ident": ident, "ones1": ones1, "onesc": onesc,
            "bias": bias_lists[b],
        }
        if shared:
            ct, st, _, _ = _rope_tables(positions[b], np.zeros(HEAD_DIM, np.float32))
            m["ct"], m["st"] = b16(ct), b16(st)
        else:
            for nm, tb in zip(("cq1", "sq1", "cq2", "sq2"),
                              _rope_tables(positions[b], q_scale)):
                m[nm] = b16(tb)
            for nm, tb in zip(("ck1", "sk1", "ck2", "sk2"),
                              _rope_tables(positions[b], k_scale)):
                m[nm] = b16(tb)
        in_maps.append(m)

    # note: all 8 cores must run the same program for SPMD; assert geometry
    # matches across batches (true for the staged problem)
    assert keys[0] == keys[1] if B == 2 else True
    nc = _prog_cache[keys[0]]

    res = run_bass_kernel_spmd(nc, in_maps, list(range(8)))
    kernel._last_results = res
    out = np.empty((B, T, T), np.float32)
    for b in range(B):
        acc = res.results[b * NUM_KV_HEADS]["yp"].astype(np.float64)
        for kh in range(1, NUM_KV_HEADS):
            acc += res.results[b * NUM_KV_HEADS + kh]["yp"].astype(np.float64)
        out[b] = acc.astype(np.float32)
    return out


# revision 25
# speedup vs baseline: 1.0276x; 1.0276x over previous
"""Trainium2 Bass kernel for nn_Attention_83081847374268 (sparse sliding-window GQA).

Sharding: 8 cores = batch (2, data parallel) x kv-head (4, tensor parallel).
Each core computes, for its (b, kh): q/k/v projections (2 q heads, 1 kv head),
QK-RMSNorm + RoPE, banded sliding-window attention, and a partial output
projection against its 512-row slice of wout.  The host sums the 4 partials
per batch (the TP reduction) and stacks the batches.

v2 design (vs the fp32r baseline):
  * all DMA'd tensors are bf16 (x, weights, rope tables, bias, output) --
    halves HBM traffic and enables fast weight loads (FWL) on the PE;
    PSUM accumulation stays fp32 so only operand rounding is lost.
  * v is computed directly in natural [t, head_dim] layout (x-chunk as the
    stationary operand) into a persistent SBUF buffer -- no DRAM bounce, no
    PE transposes for v.
  * the three stages are interleaved per 512-token chunk:
      A(chunk t) -> B(groups 2t, 2t+1) -> C(out-proj tiles 4t..4t+3)
    so the PE never idles long enough for HAM to re-throttle and all DMA
    (x in, yp out) streams concurrently with compute.
  * bias (mask) tiles are deduped host-side (interior tiles share one banded
    pattern) and kept resident in SBUF.
  * kT / v_sb are zeroed up front so tiles whose padded key window reaches
    past the currently-written columns read zeros (masked to exp(-100)=0),
    never junk/NaN.
"""
import sys

sys.path.insert(0, "/opt/trn_rl_repo")

import numpy as np
import ml_dtypes

import concourse.bacc as bacc
import concourse.mybir as mybir
from concourse.bass_utils import run_bass_kernel_spmd
from concourse.tile import TileContext
from concourse.alu_op_type import AluOpType

F32 = mybir.dt.float32
F32R = mybir.dt.float32r
BF16 = mybir.dt.bfloat16
ACTF = mybir.ActivationFunctionType

B, T, WIDTH = 2, 2048, 2048
NUM_HEADS, NUM_KV_HEADS, HEAD_DIM = 8, 4, 256
GROUPS = NUM_HEADS // NUM_KV_HEADS  # 2 q heads per kv head (= per core)
WINDOW = 512
ROPE_BASE = 10000.0
MASK_NEG = -100.0  # exp(S/16 + MASK_NEG) == 0 for |S|<=~16; exact in bf16

NT = T // 128           # 16 query tiles
TCH = 512               # stage-A t-chunk width
NTCH = T // TCH         # 4
NW = WIDTH // 128       # 16 contraction chunks

_prog_cache = {}


def _round_up(x, m):
    return (x + m - 1) // m * m


def _geometry(positions, attn_mask):
    """Per-query-tile key windows from the actual mask/positions data."""
    pos = np.asarray(positions)
    am = np.asarray(attn_mask)
    pd = pos[:, :, None].astype(np.int64) - pos[:, None, :].astype(np.int64)
    valid = am & (np.abs(pd) < WINDOW)  # [B, T, T] bool
    assert valid.any(axis=2).all(), "a query row with no valid key is unsupported"
    js = []
    whi = []
    wmax = 0
    for it in range(NT):
        cols = valid[:, it * 128:(it + 1) * 128, :].any(axis=(0, 1))
        idx = np.nonzero(cols)[0]
        j_lo, j_hi = int(idx[0]), int(idx[-1]) + 1
        j0 = (j_lo // 128) * 128
        wmax = max(wmax, j_hi - j0)
        js.append(j0)
        whi.append(j_hi)
    Wb = max(256, _round_up(wmax, 128))
    Wb = min(Wb, T)
    js = tuple(max(0, min(j, T - Wb)) for j in js)
    wext = tuple(min(Wb, _round_up(whi[it] - js[it], 128)) for it in range(NT))
    return valid, Wb, js, wext


def _s_pieces(Wb):
    """Split Wb into PSUM-bank-aligned moving pieces (<=512 fp32 per bank)."""
    out = []
    rem = Wb
    while rem > 0:
        out.append(min(512, rem))
        rem -= out[-1]
    return out


def _rope_tables(pos_b, scale):
    """cos/sin tables in [head_dim/2, T] (transposed) layout, gain folded in."""
    d = np.arange(HEAD_DIM // 2, dtype=np.float32)
    timescale = (ROPE_BASE ** (2.0 / HEAD_DIM * d)).astype(np.float32)
    rad = pos_b.astype(np.float32)[None, :] / timescale[:, None]  # [128, T]
    cos, sin = np.cos(rad).astype(np.float32), np.sin(rad).astype(np.float32)
    g1 = (1.0 + scale[:HEAD_DIM // 2]).astype(np.float32)[:, None]
    g2 = (1.0 + scale[HEAD_DIM // 2:]).astype(np.float32)[:, None]
    # o1 = a1*C1 - a2*S2 ; o2 = a2*C2 + a1*S1
    return (cos * g1, sin * g1, cos * g2, sin * g2)  # C1, S1, C2, S2


def _build(Wb, js, wext, n_bias, bias_map, shared_tables):
    nc = bacc.Bacc("TRN2", target_bir_lowering=False, debug=False, num_devices=8)

    def din(name, shape, dt):
        return nc.dram_tensor(name, shape, dt, kind="ExternalInput").ap()

    xT = din("xT", [WIDTH, T], BF16)
    wq = din("wq", [WIDTH, 512], BF16)
    wk = din("wk", [WIDTH, 256], BF16)
    wv = din("wv", [WIDTH, 256], BF16)
    wout = din("wout", [512, T], BF16)
    ident_d = din("ident", [128, 128], BF16)
    ones1_d = din("ones1", [1, 128], F32R)    # K=1 broadcast lhsT
    onesc_d = din("onesc", [128, 1], F32R)    # partition-sum lhsT
    bias_d = din("bias", [n_bias, 128, Wb], BF16)
    tab_names = ["ct", "st"] if shared_tables else [
        "cq1", "sq1", "cq2", "sq2", "ck1", "sk1", "ck2", "sk2"]
    tabs = {n: din(n, [128, T], BF16) for n in tab_names}
    yp = nc.dram_tensor("yp", [T, T], BF16, kind="ExternalOutput").ap()

    NJ = Wb // 128
    # per-group (2 query tiles) union of key chunks, and which halves exist
    groups = []
    for g in range(NT // 2):
        w0 = set(range(js[2 * g] // 128,
                       js[2 * g] // 128 + wext[2 * g] // 128))
        w1 = set(range(js[2 * g + 1] // 128,
                       js[2 * g + 1] // 128 + wext[2 * g + 1] // 128))
        groups.append([(jc, jc in w0, jc in w1) for jc in sorted(w0 | w1)])

    with TileContext(nc) as tc:
        with (
            tc.tile_pool(name="persist", bufs=1) as pp,
            tc.tile_pool(name="qk_store", bufs=1) as qkp,
            tc.tile_pool(name="wpool", bufs=1) as wp,
            tc.tile_pool(name="xpool", bufs=2) as xp,
            tc.tile_pool(name="sa", bufs=1) as sa,
            tc.tile_pool(name="sb2", bufs=2) as sb2,
            tc.tile_pool(name="ptp", bufs=2) as ptp,
            tc.tile_pool(name="outp", bufs=2) as outp,
            tc.tile_pool(name="ps_pair", bufs=2, space="PSUM") as ps_pair,
            tc.tile_pool(name="ps_aux", bufs=2, space="PSUM") as ps_aux,
            tc.tile_pool(name="ps_o", bufs=2, space="PSUM") as ps_o,
        ):
            # ---------- persistent SBUF state ----------
            qT = [qkp.tile([128, T], BF16, tag=f"qT{c}", name=f"qT{c}") for c in range(4)]
            kT = [qkp.tile([128, T], BF16, tag=f"kT{c}", name=f"kT{c}") for c in range(2)]
            v_sb = qkp.tile([128, NT * 256], BF16, tag="v_sb", name="v_sb")
            encT = [qkp.tile([128, T], BF16, tag=f"encT{c}", name=f"encT{c}")
                    for c in range(4)]
            wq_t = wp.tile([128, NW * 512], BF16)
            wk_t = wp.tile([128, NW * 256], BF16)
            wv_t = wp.tile([128, NW * 256], BF16)
            wq_r = wq.rearrange("(c p) m -> p c m", p=128)
            wk_r = wk.rearrange("(c p) m -> p c m", p=128)
            wv_r = wv.rearrange("(c p) m -> p c m", p=128)
            wq_v = wq_t[:].rearrange("p (c m) -> p c m", m=512)
            wk_v = wk_t[:].rearrange("p (c m) -> p c m", m=256)
            wv_v = wv_t[:].rearrange("p (c m) -> p c m", m=256)
            xT_r = xT.rearrange("(c p) t -> p c t", p=128)

            # first DMAs in the queue: wk + x chunk 0, finely chunked so the
            # first unit's matmuls start ASAP
            xts0 = xp.tile([128, NW * TCH], BF16, tag="xts", name="xts0")
            xv0 = xts0[:].rearrange("p (c t) -> p c t", t=TCH)
            for wc in range(4):
                nc.sync.dma_start(out=wk_v[:, wc:wc + 1], in_=wk_r[:, wc:wc + 1])
                nc.sync.dma_start(out=xv0[:, wc:wc + 1],
                                  in_=xT_r[:, wc:wc + 1, 0:TCH])
            for q4 in range(1, 4):
                nc.sync.dma_start(out=wk_v[:, q4 * 4:(q4 + 1) * 4],
                                  in_=wk_r[:, q4 * 4:(q4 + 1) * 4])
                nc.sync.dma_start(out=xv0[:, q4 * 4:(q4 + 1) * 4],
                                  in_=xT_r[:, q4 * 4:(q4 + 1) * 4, 0:TCH])
            for q4 in range(4):
                nc.sync.dma_start(out=wq_v[:, q4 * 4:(q4 + 1) * 4],
                                  in_=wq_r[:, q4 * 4:(q4 + 1) * 4])

            # PE warmup: ~4.5us of dummy matmuls on a zeroed tile gets the
            # HAM clock-gate to full rate before the real matmuls arrive.
            # The result (zeros) is written back into zero_b, which IS
            # consumed later, so the chain is not dead code.
            zero_b = pp.tile([128, 128], BF16)
            nc.any.memset(zero_b[:], 0.0)
            warm_ps = ps_o.tile([128, 128], F32, tag="t_po", name="warm_ps")
            for _ in range(42):
                nc.tensor.matmul(warm_ps[:], zero_b[:], zero_b[:],
                                 start=True, stop=True)
            nc.vector.tensor_copy(zero_b[:], warm_ps[:])

            for c in range(2):
                nc.any.memset(kT[c][:], 0.0)
            nc.any.memset(v_sb[:], 0.0)

            ident = pp.tile([128, 128], BF16)
            nc.sync.dma_start(out=ident[:], in_=ident_d[:])
            ones1 = pp.tile([1, 128], F32R)
            nc.sync.dma_start(out=ones1[:], in_=ones1_d[:])
            onesc = pp.tile([128, 1], F32R)
            nc.sync.dma_start(out=onesc[:], in_=onesc_d[:])
            epsb = pp.tile([1, 1], F32)
            nc.any.memset(epsb[:], 1e-6)
            epsbq = pp.tile([1, 1], F32)
            nc.any.memset(epsbq[:], HEAD_DIM * 1e-6)
            ones_f = pp.tile([1, 1], F32)
            nc.any.memset(ones_f[:], 1.0)
            rstdq_c = [pp.tile([128, NT], F32, tag=f"rstdq{hh}", name=f"rstdq{hh}")
                       for hh in range(2)]

            # ---------- remaining prologue loads ------
            def load_xts(tci):
                t0 = tci * TCH
                xts = xp.tile([128, NW * TCH], BF16, tag="xts", name=f"xts{tci}")
                xv = xts[:].rearrange("p (c t) -> p c t", t=TCH)
                for q4 in range(4):
                    nc.sync.dma_start(
                        out=xv[:, q4 * 4:(q4 + 1) * 4],
                        in_=xT_r[:, q4 * 4:(q4 + 1) * 4, t0:t0 + TCH],
                    )
                return xts

            tabt = {}
            for name in tab_names:
                tt = pp.tile([128, T], BF16, tag=name, name=f"tab_{name}")
                nc.sync.dma_start(out=tt[:], in_=tabs[name][:])
                tabt[name] = tt
            for q4 in range(4):
                nc.sync.dma_start(out=wv_v[:, q4 * 4:(q4 + 1) * 4],
                                  in_=wv_r[:, q4 * 4:(q4 + 1) * 4])
            bias_t = []
            for bi in range(n_bias):
                bt = pp.tile([128, Wb], BF16, tag=f"bias{bi}", name=f"bias{bi}")
                nc.sync.dma_start(out=bt[:], in_=bias_d[bi])
                bias_t.append(bt)
            wout_t = [pp.tile([128, T], BF16, tag=f"wo{c}", name=f"wo{c}")
                      for c in range(4)]
            wout_r = wout.rearrange("(c p) t -> c p t", p=128)
            for c in range(4):
                nc.sync.dma_start(out=wout_t[c][:], in_=wout_r[c])

            if shared_tables:
                q_tabs = k_tabs = ("ct", "st", "ct", "st")
            else:
                q_tabs = ("cq1", "sq1", "cq2", "sq2")
                k_tabs = ("ck1", "sk1", "ck2", "sk2")
            units = [
                (wk_t, 256, 0, k_tabs, kT, 0, None),
                (wq_t, 512, 0, q_tabs, qT, 0, 0),
                (wq_t, 512, 256, q_tabs, qT, 2, 1),
            ]

            # ---------------- stage A: projections + RMSNorm + RoPE ----------
            def stage_a(tci, xts):
                t0 = tci * TCH
                for w_t, wcols, cbase, tkeys, dest, dbase, qhead in units:
                    pspair = ps_pair.tile([128, 2 * TCH], F32, tag="pspair",
                                          name=f"pjp{tci}_{dbase}_{qhead}")
                    ps1 = pspair[:, 0:TCH]
                    ps2 = pspair[:, TCH:2 * TCH]
                    for ps, cc in ((ps1, 0), (ps2, 1)):
                        coff = cbase + cc * 128
                        for wc in range(NW):
                            nc.tensor.matmul(
                                ps,
                                w_t[:, wc * wcols + coff: wc * wcols + coff + 128],
                                xts[:, wc * TCH:(wc + 1) * TCH],
                                start=(wc == 0), stop=(wc == NW - 1),
                            )
                    sq1 = sa.tile([128, TCH], F32R, tag="sq1")
                    sq2 = sa.tile([128, TCH], F32R, tag="sq2")
                    nc.scalar.activation(sq1[:], ps1, ACTF.Square)
                    nc.scalar.activation(sq2[:], ps2, ACTF.Square)
                    psvar = ps_aux.tile([1, TCH], F32, tag="t_aux",
                                        name=f"pvar{tci}_{dbase}_{qhead}")
                    nc.tensor.matmul(psvar[:], onesc[:], sq1[:], start=True, stop=False)
                    nc.tensor.matmul(psvar[:], onesc[:], sq2[:], start=False, stop=True)
                    C1, S1, C2, S2 = (tabt[k][:, t0:t0 + TCH] for k in tkeys)
                    m1 = sa.tile([128, TCH], F32, tag="m1")
                    m2 = sa.tile([128, TCH], F32, tag="m2")
                    m3 = sa.tile([128, TCH], F32, tag="m1", name="m3t")
                    m4 = sa.tile([128, TCH], F32, tag="m2", name="m4t")
                    if qhead is None:
                        # k: apply rstd via PE broadcast, fused into rope
                        stdv = sa.tile([1, TCH], F32R, tag="stdv")
                        nc.scalar.activation(stdv[:], psvar[:], ACTF.Sqrt,
                                             scale=1.0 / HEAD_DIM, bias=epsb[:])
                        psb = ps_aux.tile([128, TCH], F32, tag="t_aux",
                                          name=f"psb{tci}")
                        nc.tensor.matmul(psb[:], ones1[:], stdv[:],
                                         start=True, stop=True)
                        rb = sa.tile([128, TCH], F32, tag="rb")
                        nc.vector.reciprocal_approx_fast(out=rb[:], in_=psb[:])
                        a1 = sa.tile([128, TCH], F32, tag="a1")
                        a2 = sa.tile([128, TCH], F32, tag="a2")
                        nc.vector.tensor_tensor(a1[:], ps1, rb[:], AluOpType.mult)
                        nc.vector.tensor_tensor(a2[:], ps2, rb[:], AluOpType.mult)
                    else:
                        # q: defer 1/std to the stage-B logits scale;
                        # transpose 4*std per 128-tile via K=1 matmuls
                        stdvf = sa.tile([1, TCH], F32, tag="stdvf")
                        nc.scalar.activation(stdvf[:], psvar[:], ACTF.Sqrt,
                                             bias=epsbq[:])
                        sq_ps = ps_aux.tile([128, TCH // 128], F32, tag="t_aux",
                                            name=f"sqps{tci}_{qhead}")
                        for s in range(TCH // 128):
                            nc.tensor.matmul(
                                sq_ps[:, s:s + 1],
                                stdvf[:, s * 128:(s + 1) * 128],
                                ones_f[:], start=True, stop=True)
                        stdq = sa.tile([128, TCH // 128], F32, tag="stdq")
                        nc.scalar.activation(stdq[:], sq_ps[:], ACTF.Copy)
                        nc.vector.reciprocal_approx_fast(
                            out=rstdq_c[qhead][:, tci * (TCH // 128):
                                               (tci + 1) * (TCH // 128)],
                            in_=stdq[:])
                        a1, a2 = ps1, ps2
                    nc.vector.tensor_tensor(m1[:], a1, C1, AluOpType.mult)
                    nc.vector.tensor_tensor(m2[:], a2, S2, AluOpType.mult)
                    nc.vector.tensor_tensor(
                        dest[dbase][:, t0:t0 + TCH], m1[:], m2[:], AluOpType.subtract)
                    nc.vector.tensor_tensor(m3[:], a2, C2, AluOpType.mult)
                    nc.vector.tensor_tensor(m4[:], a1, S1, AluOpType.mult)
                    nc.vector.tensor_tensor(
                        dest[dbase + 1][:, t0:t0 + TCH], m3[:], m4[:], AluOpType.add)
                # v in natural [t, 256] layout: x-chunk stationary, wv moving
                for s in range(TCH // 128):
                    psv = ps_o.tile([128, 256], F32, tag="t_po",
                                    name=f"psv{tci}_{s}")
                    for wc in range(NW):
                        nc.tensor.matmul(
                            psv[:],
                            xts[:, wc * TCH + s * 128: wc * TCH + (s + 1) * 128],
                            wv_t[:, wc * 256:(wc + 1) * 256],
                            start=(wc == 0), stop=(wc == NW - 1),
                        )
                    jc = tci * (TCH // 128) + s
                    nc.vector.tensor_copy(v_sb[:, jc * 256:(jc + 1) * 256], psv[:])

            # ---------------- stage B: banded attention ----------------------
            def stage_b(g):
                ginfo = groups[g]
                nj = len(ginfo)
                jc0 = ginfo[0][0]
                pts_all = ptp.tile([128, nj * 512], BF16, tag="pts", name=f"pts{g}")
                for i, (jc, inA, inB) in enumerate(ginfo):
                    for h in range(2):
                        if not inA:
                            nc.vector.tensor_copy(
                                pts_all[:, i * 512 + h * 256:
                                        i * 512 + h * 256 + 128], zero_b[:])
                        if not inB:
                            nc.vector.tensor_copy(
                                pts_all[:, i * 512 + h * 256 + 128:
                                        i * 512 + h * 256 + 256], zero_b[:])
                den2 = sb2.tile([128, 2 * 2], F32, tag="den2", name=f"den2_{g}")
                pdict = {}
                for half in range(2):
                    it = 2 * g + half
                    jst = js[it]
                    wx = wext[it]
                    nj_t = wx // 128
                    bt = bias_t[bias_map[it]]
                    t_pieces = _s_pieces(wx)
                    for h in range(2):
                        S_ps = ps_pair.tile([128, 2 * TCH], F32, tag="pspair",
                                            name=f"S{it}_{h}")
                        for cc in range(2):
                            col = 0
                            for pi, pw in enumerate(t_pieces):
                                nc.tensor.matmul(
                                    S_ps[:, pi * 512: pi * 512 + pw],
                                    qT[2 * h + cc][:, it * 128:(it + 1) * 128],
                                    kT[cc][:, jst + col: jst + col + pw],
                                    start=(cc == 0), stop=(cc == 1),
                                )
                                col += pw
                        S_b = sb2.tile([128, Wb], BF16, tag="Sb", name=f"Sb{it}_{h}")
                        col = 0
                        for pi, pw in enumerate(t_pieces):
                            nc.vector.scalar_tensor_tensor(
                                S_b[:, col:col + pw], S_ps[:, pi * 512: pi * 512 + pw],
                                rstdq_c[h][:, it:it + 1],
                                bt[:, col:col + pw],
                                AluOpType.mult, AluOpType.add)
                            col += pw
                        P_t = sb2.tile([128, Wb], BF16, tag=f"P{h}", name=f"P{it}_{h}")
                        nc.scalar.activation(
                            P_t[:, 0:wx], S_b[:, 0:wx], ACTF.Exp,
                            accum_out=den2[:, half * 2 + h: half * 2 + h + 1])
                        pdict[(half, h)] = P_t
                    for h in range(2):
                        rden = sb2.tile([128, 1], F32, tag=f"rden{h}",
                                        name=f"rden{it}_{h}")
                        nc.vector.reciprocal_approx_fast(
                            out=rden[:],
                            in_=den2[:, half * 2 + h: half * 2 + h + 1])
                        P_t = pdict[(half, h)]
                        Pn = sb2.tile([128, Wb], BF16, tag="Pn", name=f"Pn{it}_{h}")
                        nc.vector.tensor_scalar_mul(Pn[:, 0:wx], P_t[:, 0:wx],
                                                    rden[:])
                        idx0 = next(i for i, (c, _, _) in enumerate(ginfo)
                                    if c == jst // 128)
                        ps_t = ps_aux.tile([128, NJ * 128], BF16, tag="t_aux",
                                           name=f"ptps{it}_{h}")
                        for k in range(nj_t):
                            nc.tensor.transpose(
                                ps_t[:, k * 128:(k + 1) * 128],
                                Pn[:, k * 128:(k + 1) * 128],
                                ident[:])
                        pts_v = pts_all[:].rearrange(
                            "p (i f c) -> p i f c", f=4, c=128)
                        nc.vector.tensor_copy(
                            pts_v[:, idx0: idx0 + nj_t, h * 2 + half, :],
                            ps_t[:, 0:nj_t * 128].rearrange(
                                "p (k c) -> p k c", c=128))
                # PV: accumulate encoded^T from v_sb directly
                for cc in range(2):
                    eps = ps_o.tile([128, 512], F32, tag="t_po", name=f"eps{g}_{cc}")
                    for i, (jc, _, _) in enumerate(ginfo):
                        nc.tensor.matmul(
                            eps[:], v_sb[:, jc * 256 + cc * 128: jc * 256 + (cc + 1) * 128],
                            pts_all[:, i * 512:(i + 1) * 512],
                            start=(i == 0), stop=(i == nj - 1),
                        )
                    for h in range(2):
                        nc.scalar.activation(
                            encT[2 * h + cc][:, g * 256:(g + 1) * 256],
                            eps[:, h * 256:(h + 1) * 256], ACTF.Copy)

            # ---------------- stage C: output projection ----------------------
            def stage_c(tt):
                ob = outp.tile([128, T], BF16, tag="ob", name=f"ob{tt}")
                for nb in range(4):
                    ops = ps_o.tile([128, 512], F32, tag="t_po", name=f"ops{tt}_{nb}")
                    for cc in range(4):
                        nc.tensor.matmul(
                            ops[:],
                            encT[cc][:, tt * 128:(tt + 1) * 128],
                            wout_t[cc][:, nb * 512:(nb + 1) * 512],
                            start=(cc == 0), stop=(cc == 3),
                        )
                    nc.vector.tensor_copy(ob[:, nb * 512:(nb + 1) * 512], ops[:])
                    nc.sync.dma_start(
                        out=yp[tt * 128:(tt + 1) * 128, nb * 512:(nb + 1) * 512],
                        in_=ob[:, nb * 512:(nb + 1) * 512])

            # ---------------- interleaved main loop --------------------------
            # stage C trails one group so its matmuls are ready PE work while
            # the next group's softmax (DVE/ACT) chains run
            for tci in range(NTCH):
                xts = xts0 if tci == 0 else load_xts(tci)
                stage_a(tci, xts)
                for g in (2 * tci, 2 * tci + 1):
                    stage_b(g)
                    if g >= 1:
                        stage_c(2 * g - 2)
                        stage_c(2 * g - 1)
            stage_c(2 * (NT // 2) - 2)
            stage_c(2 * (NT // 2) - 1)

    nc.compile()
    return nc


def kernel(x, positions, attn_mask, wq, wkv, wout, q_scale, k_scale):
    x = np.asarray(x, np.float32)
    positions = np.asarray(positions)
    wq = np.asarray(wq, np.float32)
    wkv = np.asarray(wkv, np.float32)
    wout = np.asarray(wout, np.float32)
    q_scale = np.asarray(q_scale, np.float32)
    k_scale = np.asarray(k_scale, np.float32)

    valid, Wb, js, wext = _geometry(positions, attn_mask)
    shared = not (q_scale.any() or k_scale.any())

    # host-side bias bands: 0 where valid, MASK_NEG elsewhere (incl. padding);
    # identical tiles dedup to shared SBUF-resident tables
    bias = np.full((B, NT, 128, Wb), MASK_NEG, np.float32)
    for it in range(NT):
        j0 = js[it]
        w = min(Wb, T - j0)
        vslab = valid[:, it * 128:(it + 1) * 128, j0:j0 + w]
        bias[:, it, :, :w][vslab] = 0.0
    bias = bias.astype(ml_dtypes.bfloat16)
    bias_tabs = {}   # per batch: bytes -> idx
    bias_maps = []
    bias_lists = []
    for b in range(B):
        tab, bmap, blist = {}, [], []
        for it in range(NT):
            key = bias[b, it].tobytes()
            if key not in tab:
                tab[key] = len(blist)
                blist.append(bias[b, it])
            bmap.append(tab[key])
        bias_maps.append(tuple(bmap))
        bias_lists.append(np.stack(blist))
    # all cores of a batch share geometry; builds are keyed per distinct map
    keys = [(Wb, js, wext, len(bias_lists[b]), bias_maps[b], shared)
            for b in range(B)]
    for key in keys:
        if key not in _prog_cache:
            _prog_cache[key] = _build(*key)

    ident = np.eye(128, dtype=ml_dtypes.bfloat16)
    ones1 = np.ones((1, 128), np.float32)
    onesc = np.ones((128, 1), np.float32)

    def b16(a):
        return np.ascontiguousarray(a.astype(ml_dtypes.bfloat16))

    in_maps = []
    for core in range(8):
        b, kh = divmod(core, NUM_KV_HEADS)
        m = {
            "xT": b16(x[b].T),
            "wq": b16(wq[:, kh * 512:(kh + 1) * 512]),
            "wk": b16(wkv[:, kh * 256:(kh + 1) * 256]),
            "wv": b16(wkv[:, 1024 + kh * 256: 1024 + (kh + 1) * 256]),
            "wout": b16(wout[kh * 512:(kh + 1) * 512, :]),
            "ident": ident, "ones1": ones1, "onesc": onesc,
            "bias": bias_lists[b],
        }
        if shared:
            ct, st, _, _ = _rope_tables(positions[b], np.zeros(HEAD_DIM, np.float32))
            m["ct"], m["st"] = b16(ct), b16(st)
        else:
            for nm, tb in zip(("cq1", "sq1", "cq2", "sq2"),
                              _rope_tables(positions[b], q_scale)):
                m[nm] = b16(tb)
            for nm, tb in zip(("ck1", "sk1", "ck2", "sk2"),
                              _rope_tables(positions[b], k_scale)):
                m[nm] = b16(tb)
        in_maps.append(m)

    # note: all 8 cores must run the same program for SPMD; assert geometry
    # matches across batches (true for the staged problem)
    assert keys[0] == keys[1] if B == 2 else True
    nc = _prog_cache[keys[0]]

    res = run_bass_kernel_spmd(nc, in_maps, list(range(8)))
    kernel._last_results = res
    out = np.empty((B, T, T), np.float32)
    for b in range(B):
        acc = res.results[b * NUM_KV_HEADS]["yp"].astype(np.float64)
        for kh in range(1, NUM_KV_HEADS):
            acc += res.results[b * NUM_KV_HEADS + kh]["yp"].astype(np.float64)
        out[b] = acc.astype(np.float32)
    return out
